# revision 8
# baseline (speedup 1.0000x reference)
"""Positional-encoding broadcast kernel for Trainium2 (8 NeuronCores).

The reference builds the interleaved sin/cos PE table [4096, 2048] f32 and
broadcasts it to [32, 4096, 2048] -- a 1 GiB, purely memory-bound output.
Sharding: by sequence.  Core i owns rows [512*i, 512*(i+1)) (a 4 MiB PE
slice, computed on host bit-identically to the reference's f32 jax-on-CPU
math) and writes those rows for all 32 batches = 128 MiB of HBM writes per
core.

v2: the v1 trace showed the store stream is NOT HBM-limited but SDMA-engine
limited: all 16 SDMA engines got equal work at their ~26.8 GB/s ceiling,
except SDMA engine 15 which runs ~19% slower (the known 7/15-slow pitfall)
and straggled the kernel by ~70 us.  Engine k serves a fixed set of SBUF
partitions (port map: port = ((p>>2)&7)<<1 | ((p>>6)&1)); port 15 owns
partitions {92-95, 124-127}.  Fix: those partitions keep only K=26 of the
32 batch-copies of their PE rows; the other 6 copies are stored from
duplicate copies of those rows parked on fast-port partitions (0..15 and
64..79 -> ports 0-7).

Device program (raw Bass; walrus build allows 1 sync-wait per instruction):
- SBUF layout r=2: tile[p, c*4096 + r*2048 + m] = pe[c*256 + 2p + r, m],
  so each store descriptor covers 16 KiB contiguous DRAM.  dup[q, m] holds
  the 32 port-15-owned rows (8 KiB descriptors for their tail stores).
- All loads + stores chained on the sync HWDGE ring: per-SDMA-engine FIFO
  orders store reads after load writes with no semaphore round-trip.
  Chunk 1 loads concurrently on the scalar HWDGE ring (sem-guarded).
- Every sync-ring DMA incs ring_sem by 16 (walrus requires sync info on
  every HWDGE DMA; the DGE distributes the total across participating
  engines), and the kernel ends on wait_ge(ring_sem, 16 * n_dmas).
"""

import numpy as np

B = 32
SEQ = 4096
D = 2048
N_CORES = 8
S_SHARD = SEQ // N_CORES          # 512
NCH = 2                           # chunks of 256 rows
R = 2                             # rows per partition
CW = R * D                        # 4096
K = 26                            # batch-copies kept on port-15 partitions

# Port-15 partitions {92..95, 124..127} own rows {2p, 2p+1} of each chunk:
# local rows {184..191, 248..255} (c0) and {440..447, 504..511} (c1).
DUP_ROWS = (
    list(range(184, 192)) + list(range(248, 256))
    + list(range(440, 448)) + list(range(504, 512))
)
# dup row j lives on SBUF partition DUP_PARTS[j] (ports 0-7 only).
DUP_PART_BASE = (0, 8, 64, 72)    # groups of 8 rows -> partition ranges

_cache = {}


def _pe_table() -> np.ndarray:
    import jax
    import jax.numpy as jnp

    cpu = jax.devices("cpu")[0]
    with jax.default_device(cpu):
        n = 10000.0
        pos = jnp.arange(SEQ, dtype=jnp.float32)[:, None]
        i = jnp.arange(D // 2, dtype=jnp.float32)[None, :]
        theta = pos / jnp.power(n, (2.0 * i) / D)
        pe = jnp.stack([jnp.sin(theta), jnp.cos(theta)], axis=-1)
        pe = pe.reshape(SEQ, D)
        return np.asarray(jax.device_get(pe))


def build_nc():
    import concourse.bass as bass
    import concourse.mybir as mybir

    nc = bass.Bass()
    pe_in = nc.dram_tensor("pe", [S_SHARD, D], mybir.dt.float32, kind="ExternalInput")
    dup_in = nc.dram_tensor("dup", [32, D], mybir.dt.float32, kind="ExternalInput")
    out = nc.dram_tensor(
        "out", [B, S_SHARD, D], mybir.dt.float32, kind="ExternalOutput"
    )
    with (
        nc.sbuf_tensor([128, NCH * CW], mybir.dt.float32) as tile,
        nc.sbuf_tensor([128, D], mybir.dt.float32) as dup,
        nc.semaphore("load_sem") as load_sem,
        nc.semaphore("ld_sync") as ld_sync,
        nc.semaphore("ring_sem") as ring_sem,
        nc.Block() as block,
    ):
        pe_src = pe_in.rearrange("(c p r) m -> p c (r m)", c=NCH, p=128, r=R)
        tile_c = tile[:].rearrange("p (c x) -> p c x", c=NCH)

        def bcast_src(c, p0, p1, nb):
            return (
                tile[p0:p1, c * CW : (c + 1) * CW]
                .unsqueeze(1)
                .broadcast_to([p1 - p0, nb, CW])
            )

        def dst_rows(c, b0, b1, row0, nrows):
            return out[
                b0:b1,
                c * 256 + row0 : c * 256 + row0 + nrows,
                :,
            ].rearrange("b (p r) m -> p b (r m)", p=nrows // R, r=R)

        def dup_store(sync, c, g):
            # group g of chunk c: 8 rows on partitions [pb, pb+8)
            pb = DUP_PART_BASE[2 * c + g]
            row0 = (184, 248)[g]
            dst = out[K:B, c * 256 + row0 : c * 256 + row0 + 8, :].rearrange(
                "b j m -> j b m"
            )
            src = dup[pb : pb + 8, :].unsqueeze(1).broadcast_to([8, B - K, D])
            sync.dma_start(out=dst, in_=src).then_inc(ring_sem, 16)

        @block.scalar
        def _(scalar):
            scalar.dma_start(out=tile_c[:, 1, :], in_=pe_src[:, 1, :]).then_inc(
                load_sem, 16
            )

        @block.sync
        def _(sync):
            # Chunk-0 + dup loads.  Load descriptors are sprayed across all
            # 16 engines (not port-affine), so only the full-width h0/h1a
            # stores may rely on same-ring FIFO ordering (baseline-proven);
            # subset and dup stores are sem-guarded below.
            sync.dma_start(out=tile_c[:, 0, :], in_=pe_src[:, 0, :]).then_inc(
                ld_sync, 16
            )
            sync.dma_start(out=dup[0:16, :], in_=dup_in[0:16, :]).then_inc(ld_sync, 16)
            sync.dma_start(out=dup[64:80, :], in_=dup_in[16:32, :]).then_inc(
                ld_sync, 16
            )

            def full_stores(c):
                sync.dma_start(out=dst_rows(c, 16, K, 0, 256), in_=bcast_src(c, 0, 128, K - 16)).then_inc(ring_sem, 16)
                sync.dma_start(out=dst_rows(c, 0, 16, 0, 256), in_=bcast_src(c, 0, 128, 16)).then_inc(ring_sem, 16)

            def tail_stores(c):
                # tail batches [K,32): rows 0..183 and 192..247 from home
                # partitions (ports 0-14); port-15 rows via dup stores.
                sync.dma_start(out=dst_rows(c, K, B, 0, 184), in_=bcast_src(c, 0, 92, B - K)).then_inc(ring_sem, 16)
                sync.dma_start(out=dst_rows(c, K, B, 192, 56), in_=bcast_src(c, 96, 124, B - K)).then_inc(ring_sem, 16)

            full_stores(0)
            sync.wait_ge(ld_sync, 48)
            tail_stores(0)
            dup_store(sync, 0, 0)
            dup_store(sync, 0, 1)
            dup_store(sync, 1, 0)
            dup_store(sync, 1, 1)
            sync.wait_ge(load_sem, 16)
            full_stores(1)
            tail_stores(1)
            sync.wait_ge(ring_sem, 16 * 12)

    return nc


def make_in_maps(pe: np.ndarray):
    maps = []
    for i in range(N_CORES):
        sl = pe[i * S_SHARD : (i + 1) * S_SHARD]
        maps.append({"pe": sl, "dup": np.ascontiguousarray(sl[DUP_ROWS])})
    return maps


def kernel(x: np.ndarray) -> np.ndarray:
    from concourse.bass_utils import run_bass_kernel_spmd

    assert x.shape[0] == B

    pe = _pe_table()
    if "nc" not in _cache:
        _cache["nc"] = build_nc()
    res = run_bass_kernel_spmd(_cache["nc"], make_in_maps(pe), list(range(N_CORES)))
    outs = [res.results[i]["out"] for i in range(N_CORES)]
    return np.concatenate(outs, axis=1)


# revision 9
# speedup vs baseline: 1.2396x; 1.2396x over previous
"""Positional-encoding broadcast kernel for Trainium2 (8 NeuronCores).

The reference builds the interleaved sin/cos PE table [4096, 2048] f32 and
broadcasts it to [32, 4096, 2048] -- a 1 GiB, purely memory-bound output.
Sharding: by sequence.  Core i owns rows [512*i, 512*(i+1)) (a 4 MiB PE
slice, computed on host bit-identically to the reference's f32 jax-on-CPU
math) and writes those rows for all 32 batches = 128 MiB of HBM writes per
core.

Perf model (from NTFF traces): the store stream is SDMA-engine limited,
not HBM limited.  Walrus splits each DMACopy's partition dim P over
n = (largest divisor of P that is <= 16) engines, always engines 0..n-1,
P/n contiguous partitions each, ~26.8 GB/s per engine.  SDMA engine 15 is
~19% slower (known 7/15-slow erratum) and straggles a uniform split by
~70 us.  Skew: engine 15 (partitions 120..127 in every 128-partition
transfer) keeps only K=26 of the 32 batch-copies:

- A(c): [128 parts, K b]   -> 16 engines, 208 pkts each
- B(c): [parts 0..119, 32-K b] -> 15 engines (no engine 15), 48 pkts each
- C(c): [parts 120..127, 32-K b] -> engines 0..7, 6 pkts each
  (engines may read any partition; C is sem-guarded because its engines
  did not execute the load descriptors for partitions 120..127)

Device program (raw Bass; walrus build allows 1 sync-wait per instruction):
- SBUF layout r=2: tile[p, c*4096 + r*2048 + m] = pe[c*256 + 2p + r, m],
  so each store descriptor covers 16 KiB contiguous DRAM.
- Chunk-0 load + A(c0) chained on the sync HWDGE ring: per-engine FIFO
  (identical partition->engine split) orders store reads after load
  writes with no semaphore round-trip.  Chunk 1 loads concurrently on the
  scalar HWDGE ring; its stores follow a load_sem wait.
"""

import numpy as np

B = 32
SEQ = 4096
D = 2048
N_CORES = 8
S_SHARD = SEQ // N_CORES          # 512
NCH = 2                           # chunks of 256 rows
R = 2                             # rows per partition
CW = R * D                        # 4096
K = 26                            # batch-copies kept on engine-15 partitions

_cache = {}


def _pe_table() -> np.ndarray:
    import jax
    import jax.numpy as jnp

    cpu = jax.devices("cpu")[0]
    with jax.default_device(cpu):
        n = 10000.0
        pos = jnp.arange(SEQ, dtype=jnp.float32)[:, None]
        i = jnp.arange(D // 2, dtype=jnp.float32)[None, :]
        theta = pos / jnp.power(n, (2.0 * i) / D)
        pe = jnp.stack([jnp.sin(theta), jnp.cos(theta)], axis=-1)
        pe = pe.reshape(SEQ, D)
        return np.asarray(jax.device_get(pe))


def build_nc():
    import concourse.bass as bass
    import concourse.mybir as mybir

    nc = bass.Bass()
    pe_in = nc.dram_tensor("pe", [S_SHARD, D], mybir.dt.float32, kind="ExternalInput")
    out = nc.dram_tensor(
        "out", [B, S_SHARD, D], mybir.dt.float32, kind="ExternalOutput"
    )
    with (
        nc.sbuf_tensor([128, NCH * CW], mybir.dt.float32) as tile,
        nc.semaphore("load_sem") as load_sem,
        nc.semaphore("ld_sync") as ld_sync,
        nc.semaphore("ring_sem") as ring_sem,
        nc.Block() as block,
    ):
        pe_src = pe_in.rearrange("(c p r) m -> p c (r m)", c=NCH, p=128, r=R)
        tile_c = tile[:].rearrange("p (c x) -> p c x", c=NCH)

        def bcast_src(c, p0, p1, nb):
            return (
                tile[p0:p1, c * CW : (c + 1) * CW]
                .unsqueeze(1)
                .broadcast_to([p1 - p0, nb, CW])
            )

        def dst(c, b0, b1, p0, p1):
            return out[
                b0:b1,
                c * 256 + 2 * p0 : c * 256 + 2 * p1,
                :,
            ].rearrange("b (p r) m -> p b (r m)", p=p1 - p0, r=R)

        @block.scalar
        def _(scalar):
            scalar.dma_start(out=tile_c[:, 1, :], in_=pe_src[:, 1, :]).then_inc(
                load_sem, 16
            )

        @block.sync
        def _(sync):
            sync.dma_start(out=tile_c[:, 0, :], in_=pe_src[:, 0, :]).then_inc(
                ld_sync, 16
            )
            # A(c0): FIFO-ordered behind the chunk-0 load (same split).
            sync.dma_start(out=dst(0, 0, K, 0, 128), in_=bcast_src(0, 0, 128, K)).then_inc(ring_sem, 16)
            sync.wait_ge(ld_sync, 16)
            sync.dma_start(out=dst(0, K, B, 0, 120), in_=bcast_src(0, 0, 120, B - K)).then_inc(ring_sem, 16)
            sync.dma_start(out=dst(0, K, B, 120, 128), in_=bcast_src(0, 120, 128, B - K)).then_inc(ring_sem, 16)
            sync.wait_ge(load_sem, 16)
            sync.dma_start(out=dst(1, 0, K, 0, 128), in_=bcast_src(1, 0, 128, K)).then_inc(ring_sem, 16)
            sync.dma_start(out=dst(1, K, B, 0, 120), in_=bcast_src(1, 0, 120, B - K)).then_inc(ring_sem, 16)
            sync.dma_start(out=dst(1, K, B, 120, 128), in_=bcast_src(1, 120, 128, B - K)).then_inc(ring_sem, 16)
            sync.wait_ge(ring_sem, 16 * 6)

    return nc


def make_in_maps(pe: np.ndarray):
    return [{"pe": pe[i * S_SHARD : (i + 1) * S_SHARD]} for i in range(N_CORES)]


def kernel(x: np.ndarray) -> np.ndarray:
    from concourse.bass_utils import run_bass_kernel_spmd

    assert x.shape[0] == B

    pe = _pe_table()
    if "nc" not in _cache:
        _cache["nc"] = build_nc()
    res = run_bass_kernel_spmd(_cache["nc"], make_in_maps(pe), list(range(N_CORES)))
    outs = [res.results[i]["out"] for i in range(N_CORES)]
    return np.concatenate(outs, axis=1)


# revision 11
# speedup vs baseline: 1.3366x; 1.0783x over previous
"""Positional-encoding broadcast kernel for Trainium2 (8 NeuronCores).

The reference builds the interleaved sin/cos PE table [4096, 2048] f32 and
broadcasts it to [32, 4096, 2048] -- a 1 GiB, purely memory-bound output.
Sharding: by sequence.  Core i owns rows [512*i, 512*(i+1)) (a 4 MiB PE
slice, computed on host bit-identically to the reference's f32 jax-on-CPU
math) and writes those rows for all 32 batches = 128 MiB of HBM writes per
core.

Perf model (from NTFF traces): the store stream is SDMA-engine limited,
not HBM limited.  Walrus splits each DMACopy's partition dim P over
n = (largest divisor of P that is <= 16) engines, always engines 0..n-1,
P/n contiguous partitions each, ~26.8 GB/s per engine.  CRITICAL: only
P=128 transfers get a clean balanced assignment; P=120/92/28/8 stores
measured at 1/2 to 1/8 of the per-engine rate (bad engine/port layout),
so every store here is exactly 128 partitions.

Device program (raw Bass; walrus build allows 1 sync-wait per instruction):
- SBUF layout r=2: tile[p, c*4096 + r*2048 + m] = pe[c*256 + 2p + r, m],
  so each store descriptor covers 16 KiB contiguous DRAM.
- Chunk-0 load + A(c0) chained on the sync HWDGE ring: per-engine FIFO
  (identical partition->engine split) orders store reads after load
  writes with no semaphore round-trip.  Chunk 1 loads concurrently on the
  scalar HWDGE ring; its stores follow a load_sem wait.
"""

import numpy as np

B = 32
SEQ = 4096
D = 2048
N_CORES = 8
S_SHARD = SEQ // N_CORES          # 512
NCH = 2                           # chunks of 256 rows
R = 2                             # rows per partition
CW = R * D                        # 4096
K = 26                            # batch-copies kept on engine-15 partitions

_cache = {}


def _pe_table() -> np.ndarray:
    import jax
    import jax.numpy as jnp

    cpu = jax.devices("cpu")[0]
    with jax.default_device(cpu):
        n = 10000.0
        pos = jnp.arange(SEQ, dtype=jnp.float32)[:, None]
        i = jnp.arange(D // 2, dtype=jnp.float32)[None, :]
        theta = pos / jnp.power(n, (2.0 * i) / D)
        pe = jnp.stack([jnp.sin(theta), jnp.cos(theta)], axis=-1)
        pe = pe.reshape(SEQ, D)
        return np.asarray(jax.device_get(pe))


def build_nc():
    import concourse.bass as bass
    import concourse.mybir as mybir

    nc = bass.Bass()
    pe_in = nc.dram_tensor("pe", [S_SHARD, D], mybir.dt.float32, kind="ExternalInput")
    out = nc.dram_tensor(
        "out", [B, S_SHARD, D], mybir.dt.float32, kind="ExternalOutput"
    )
    with (
        nc.sbuf_tensor([128, NCH * CW], mybir.dt.float32) as tile,
        nc.semaphore("load_sem") as load_sem,
        nc.semaphore("ld_sync") as ld_sync,
        nc.semaphore("ring_sem") as ring_sem,
        nc.Block() as block,
    ):
        pe_src = pe_in.rearrange("(c p r) m -> p c (r m)", c=NCH, p=128, r=R)
        tile_c = tile[:].rearrange("p (c x) -> p c x", c=NCH)

        def bcast_src(c, p0, p1, nb):
            return (
                tile[p0:p1, c * CW : (c + 1) * CW]
                .unsqueeze(1)
                .broadcast_to([p1 - p0, nb, CW])
            )

        def dst(c, b0, b1, p0, p1):
            return out[
                b0:b1,
                c * 256 + 2 * p0 : c * 256 + 2 * p1,
                :,
            ].rearrange("b (p r) m -> p b (r m)", p=p1 - p0, r=R)

        @block.scalar
        def _(scalar):
            scalar.dma_start(out=tile_c[:, 1, :], in_=pe_src[:, 1, :]).then_inc(
                load_sem, 16
            )

        @block.sync
        def _(sync):
            sync.dma_start(out=tile_c[:, 0, :], in_=pe_src[:, 0, :]).then_inc(
                ld_sync, 16
            )
            # A(c0): FIFO-ordered behind the chunk-0 load (same split).
            sync.dma_start(out=dst(0, 0, K, 0, 128), in_=bcast_src(0, 0, 128, K)).then_inc(ring_sem, 16)
            sync.wait_ge(ld_sync, 16)
            sync.dma_start(out=dst(0, K, B, 0, 128), in_=bcast_src(0, 0, 128, B - K)).then_inc(ring_sem, 16)
            sync.wait_ge(load_sem, 16)
            sync.dma_start(out=dst(1, 0, K, 0, 128), in_=bcast_src(1, 0, 128, K)).then_inc(ring_sem, 16)
            sync.dma_start(out=dst(1, K, B, 0, 128), in_=bcast_src(1, 0, 128, B - K)).then_inc(ring_sem, 16)
            sync.wait_ge(ring_sem, 16 * 4)

    return nc


def make_in_maps(pe: np.ndarray):
    return [{"pe": pe[i * S_SHARD : (i + 1) * S_SHARD]} for i in range(N_CORES)]


def kernel(x: np.ndarray) -> np.ndarray:
    from concourse.bass_utils import run_bass_kernel_spmd

    assert x.shape[0] == B

    pe = _pe_table()
    if "nc" not in _cache:
        _cache["nc"] = build_nc()
    res = run_bass_kernel_spmd(_cache["nc"], make_in_maps(pe), list(range(N_CORES)))
    outs = [res.results[i]["out"] for i in range(N_CORES)]
    return np.concatenate(outs, axis=1)


# revision 29
# speedup vs baseline: 1.5848x; 1.1857x over previous
"""Positional-encoding broadcast kernel for Trainium2 (8 NeuronCores).

The reference builds the interleaved sin/cos PE table [4096, 2048] f32 and
broadcasts it to [32, 4096, 2048] -- a 1 GiB, purely memory-bound output.
Sharding: by sequence.  Core i owns rows [512*i, 512*(i+1)) and writes
those rows for all 32 batches = 128 MiB of HBM writes per core.

Perf model (from NTFF traces): per-SDMA-engine ceiling ~26.8 GB/s, 16
engines, fabric ~435 GB/s/core; when sibling cores run fully overlapped
the HBM stack share (~716/2 GB/s) binds instead (~722 ns vs 610 ns per
16 KiB packet).  Walrus splits each DMACopy's partition dim P over
n = (largest divisor of P <= 16) engines (0..n-1, contiguous blocks);
only P=128 gets a clean balanced assignment (P=120/92/28/8 measured at
1/2 to 1/8 rate), so every store here is exactly 128 partitions.

To cut HBM read traffic (it steals stack bandwidth from the stores),
only chunk 0 (rows 0..255) is loaded from DRAM (2 MiB); chunk 1 is
computed on-device (abs err ~1e-3, gate is 2e-2):
  u   = pos_p * freq2_k (+0.25 for cos)      pos integer-exact in f32,
                                             freq2 = 1/(2pi*10000^(k/1024))
                                             a host-replicated 512 KiB input
                                             (ACT Exp's ~7e-6 rel err scales
                                             with theta<=4095 -> 2.7e-2, too
                                             big, so no on-device exp)
  y   = u - rne_int_cast(u)  in [-.5, .5]    (DVE f32->i32 is RNE on HW)
  ACT sin(2pi*y) = sin/cos(theta)
interleaved directly into the chunk-1 SBUF region with stride-2 writes.
(AluOpType.mod is rejected by the TensorScalar ISA check; the int-cast
frac extraction is the supported path.)

Device program (raw Bass; walrus build allows 1 sync-wait per
instruction): SBUF layout r=2: tile[p, c*4096 + r*2048 + m] =
pe[c*256 + 2p + r, m] -> 16 KiB contiguous DRAM per store descriptor.
Chunk-0 load + its stores chained on the sync HWDGE ring (per-engine
FIFO orders store reads after load writes); chunk-1 stores wait on the
compute-done semaphore.
"""

import math

import numpy as np

B = 32
SEQ = 4096
D = 2048
N_CORES = 8
S_SHARD = SEQ // N_CORES          # 512
NCH = 2                           # chunks of 256 rows
R = 2                             # rows per partition
CW = R * D                        # 4096
K = 26                            # b-split of the stores (2 per chunk)

PI = math.pi
TWO_PI = 2.0 * math.pi
LN_N = math.log(10000.0)

_cache = {}


def _pe_table() -> np.ndarray:
    import jax
    import jax.numpy as jnp

    cpu = jax.devices("cpu")[0]
    with jax.default_device(cpu):
        n = 10000.0
        pos = jnp.arange(SEQ, dtype=jnp.float32)[:, None]
        i = jnp.arange(D // 2, dtype=jnp.float32)[None, :]
        theta = pos / jnp.power(n, (2.0 * i) / D)
        pe = jnp.stack([jnp.sin(theta), jnp.cos(theta)], axis=-1)
        pe = pe.reshape(SEQ, D)
        return np.asarray(jax.device_get(pe))


def build_nc():
    import concourse.bass as bass
    import concourse.mybir as mybir

    f32 = mybir.dt.float32
    nc = bass.Bass()
    pe_in = nc.dram_tensor("pe", [256, D], f32, kind="ExternalInput")
    pos_in = nc.dram_tensor("pos", [128, 4], f32, kind="ExternalInput")
    freq2_in = nc.dram_tensor("freq2", [128, 1024], f32, kind="ExternalInput")
    out = nc.dram_tensor("out", [B, S_SHARD, D], f32, kind="ExternalOutput")
    with (
        nc.sbuf_tensor([128, NCH * CW], f32) as tile,
        nc.sbuf_tensor([128, 1024], mybir.dt.int32) as kidx,
        nc.sbuf_tensor([128, 1024], f32) as freq,
        nc.sbuf_tensor([128, 1024], f32) as th,
        nc.sbuf_tensor([128, 1024], f32) as kf,
        nc.sbuf_tensor([128, 4 * 1024], f32) as wrap,
        nc.sbuf_tensor([128, 4], f32) as pos,
        nc.semaphore("ld_sync") as ld_sync,
        nc.semaphore("cs") as cs,
        nc.semaphore("c1_done") as c1_done,
        nc.semaphore("ring_sem") as ring_sem,
        nc.Block() as block,
    ):
        pe_src = pe_in.rearrange("(p r) m -> p (r m)", p=128, r=R)

        def bcast_src(c, nb):
            return (
                tile[:, c * CW : (c + 1) * CW]
                .unsqueeze(1)
                .broadcast_to([128, nb, CW])
            )

        def dst(c, b0, b1):
            return out[
                b0:b1, c * 256 : (c + 1) * 256, :
            ].rearrange("b (p r) m -> p b (r m)", p=128, r=R)

        def trig_views(r):
            seg = tile[:, CW + r * D : CW + (r + 1) * D]
            v = seg.rearrange("p (k two) -> p two k", two=2)
            return v[:, 0, :], v[:, 1, :]

        @block.vector
        def _(vector):
            st = vector.tensor_scalar
            # Waiting on the loads BEFORE the first cs inc also fences
            # ACT's bias reads (pos[:,2:3]) transitively.
            vector.wait_ge(ld_sync, 48)
            for r in range(2):
                posA = pos[:, r : r + 1]  # 512*core + 2p + 256 + r (exact)
                for trig in range(2):  # 0: sin, 1: cos (+0.25 turn)
                    w = wrap[:, (2 * r + trig) * 1024 : (2 * r + trig + 1) * 1024]
                    if trig:
                        st(th[:, :], freq[:, :], posA, 0.25, mybir.AluOpType.mult, mybir.AluOpType.add)
                    else:
                        st(th[:, :], freq[:, :], posA, None, mybir.AluOpType.mult)
                    vector.tensor_copy(out=kidx[:, :], in_=th[:, :])
                    vector.tensor_copy(out=kf[:, :], in_=kidx[:, :])
                    vector.tensor_tensor(
                        out=w, in0=th[:, :], in1=kf[:, :], op=mybir.AluOpType.subtract
                    ).then_inc(cs, 1)

        @block.scalar
        def _(scalar):
            for r in range(2):
                ev, od = trig_views(r)
                scalar.wait_ge(cs, 1 + 2 * r)
                scalar.activation(
                    ev, wrap[:, 2 * r * 1024 : (2 * r + 1) * 1024],
                    mybir.ActivationFunctionType.Sin, bias=pos[:, 2:3],
                    scale=TWO_PI,
                ).then_inc(c1_done, 1)
                scalar.wait_ge(cs, 2 + 2 * r)
                scalar.activation(
                    od, wrap[:, (2 * r + 1) * 1024 : (2 * r + 2) * 1024],
                    mybir.ActivationFunctionType.Sin, bias=pos[:, 2:3],
                    scale=TWO_PI,
                ).then_inc(c1_done, 1)

        @block.sync
        def _(sync):
            sync.dma_start(out=tile[:, 0:CW], in_=pe_src).then_inc(ld_sync, 16)
            sync.dma_start(out=pos[:, :], in_=pos_in[:, :]).then_inc(ld_sync, 16)
            sync.dma_start(out=freq[:, :], in_=freq2_in[:, :]).then_inc(ld_sync, 16)
            # chunk-0 stores: FIFO-ordered behind the chunk-0 load.
            sync.dma_start(out=dst(0, 0, K), in_=bcast_src(0, K)).then_inc(ring_sem, 16)
            sync.wait_ge(ld_sync, 48)
            sync.dma_start(out=dst(0, K, B), in_=bcast_src(0, B - K)).then_inc(ring_sem, 16)
            sync.wait_ge(c1_done, 4)
            sync.dma_start(out=dst(1, 0, K), in_=bcast_src(1, K)).then_inc(ring_sem, 16)
            sync.dma_start(out=dst(1, K, B), in_=bcast_src(1, B - K)).then_inc(ring_sem, 16)
            sync.wait_ge(ring_sem, 16 * 4)

    return nc


def make_in_maps(pe: np.ndarray):
    maps = []
    for i in range(N_CORES):
        pos = np.zeros((128, 4), dtype=np.float32)
        base = 512.0 * i + 2.0 * np.arange(128, dtype=np.float64)
        pos[:, 0] = (base + 256.0).astype(np.float32)
        pos[:, 1] = (base + 257.0).astype(np.float32)
        k = np.arange(1024, dtype=np.float64)
        freq2 = (
            1.0 / (2.0 * np.pi * np.power(10000.0, k / 1024.0))
        ).astype(np.float32)
        maps.append(
            {
                "pe": np.ascontiguousarray(pe[i * S_SHARD : i * S_SHARD + 256]),
                "pos": pos,
                "freq2": np.ascontiguousarray(np.tile(freq2, (128, 1))),
            }
        )
    return maps


def kernel(x: np.ndarray) -> np.ndarray:
    from concourse.bass_utils import run_bass_kernel_spmd

    assert x.shape[0] == B

    pe = _pe_table()
    if "nc" not in _cache:
        _cache["nc"] = build_nc()
    res = run_bass_kernel_spmd(_cache["nc"], make_in_maps(pe), list(range(N_CORES)))
    outs = [res.results[i]["out"] for i in range(N_CORES)]
    return np.concatenate(outs, axis=1)
